# revision 1
# baseline (speedup 1.0000x reference)
"""Distributed Bass kernel: 3D windowed attention with decomposed rel-pos bias.

Sharding: 8 cores = 4 batches x 2 head-groups (6 heads each).
Per-core layout is fully transposed ([channel, token]); the rel-pos bias is
folded into the scores matmul as 36 extra contraction channels (one-hot
k-position rows in the stationary operand, F = q.R tables in the moving
operand).  Softmax runs max-free (scores are O(+-8)); the denominator comes
free as a ones-row appended to V in the AV matmul.  All matmuls run in bf16
with fp32 PSUM accumulation.
"""

import os
import sys

import numpy as np

sys.path.insert(0, "/opt/trn_rl_repo")

B, D, H, W, C = 4, 8, 14, 14, 768
NH, HD = 12, 64
N = D * H * W  # 1568
HPC = 6  # heads per core
SCALE = HD ** -0.5
NKC = C // 128  # 6 k-chunks of input channels
NKT = (N + 127) // 128  # 13 token tiles (12x128 + 32)
FCH = [(0, 512), (512, 512), (1024, 512), (1536, 32)]  # free-dim chunks of N
KAUG = 100  # 64 qk channels + 8 + 14 + 14 bias channels

_CACHED = {}


def _build_nc():
    import concourse.bass as bass  # noqa: F401
    import concourse.mybir as mybir
    import concourse.tile as tile
    from concourse import bacc

    f32 = mybir.dt.float32
    bf16 = mybir.dt.bfloat16
    AF = mybir.ActivationFunctionType

    dbg = bool(int(os.environ.get("KERNEL_DEBUG", "0")))
    nc = bacc.Bacc(None, target_bir_lowering=False)

    # --- DRAM parameters (per-core shards; host pre-transposes/reorders) ---
    xT_d = nc.declare_dram_parameter("xT", [NKC, 128, N], bf16, isOutput=False)
    wqkv_d = nc.declare_dram_parameter("wqkv", [NKC, 128, 1152], bf16, isOutput=False)
    wproj_d = nc.declare_dram_parameter("wproj", [3, 128, 768], bf16, isOutput=False)
    oneh_d = nc.declare_dram_parameter("oneh", [36, N], bf16, isOutput=False)
    rdT_d = nc.declare_dram_parameter("rdT", [128, D * 40], bf16, isOutput=False)
    rhT_d = nc.declare_dram_parameter("rhT", [128, H * 46], bf16, isOutput=False)
    rwT_d = nc.declare_dram_parameter("rwT", [128, W * 46], bf16, isOutput=False)
    bqk_d = nc.declare_dram_parameter("bqk", [128, 6], f32, isOutput=False)
    bv_d = nc.declare_dram_parameter("bv", [64, 6], f32, isOutput=False)
    out_d = nc.declare_dram_parameter("out", [NKC, 128, N], f32, isOutput=True)
    if dbg:
        dq_d = nc.declare_dram_parameter("dbg_q", [KAUG, N], bf16, isOutput=True)
        dk_d = nc.declare_dram_parameter("dbg_k", [KAUG, N], bf16, isOutput=True)
        dv_d = nc.declare_dram_parameter("dbg_v", [128, NKT * HPC * 65], bf16, isOutput=True)
        de_d = nc.declare_dram_parameter("dbg_e", [128, N], bf16, isOutput=True)
        dr_d = nc.declare_dram_parameter("dbg_r", [128, N], f32, isOutput=True)
        da_d = nc.declare_dram_parameter("dbg_a", [128, N], bf16, isOutput=True)

    with tile.TileContext(nc) as tc:
        with (
            tc.tile_pool(name="const", bufs=1) as cpool,
            tc.tile_pool(name="work", bufs=2) as wpool,
            tc.tile_pool(name="psum", bufs=4, space="PSUM") as mmp,
            tc.tile_pool(name="psav", bufs=1, space="PSUM") as avp,
        ):
            # ---- load constants ----
            xT = cpool.tile([128, NKC * N], bf16)
            wqkv = cpool.tile([128, NKC * 1152], bf16)
            wproj = cpool.tile([128, 3 * 768], bf16)
            oneh = cpool.tile([36, N], bf16)
            rdT = cpool.tile([128, D * 40], bf16)
            rhT = cpool.tile([128, H * 46], bf16)
            rwT = cpool.tile([128, W * 46], bf16)
            bqk = cpool.tile([128, 6], f32)
            bv = cpool.tile([64, 6], f32)
            for kc in range(NKC):
                nc.sync.dma_start(xT[:, kc * N:(kc + 1) * N], xT_d[kc])
                nc.sync.dma_start(wqkv[:, kc * 1152:(kc + 1) * 1152], wqkv_d[kc])
            for t3 in range(3):
                nc.sync.dma_start(wproj[:, t3 * 768:(t3 + 1) * 768], wproj_d[t3])
            nc.sync.dma_start(oneh[:], oneh_d[:])
            nc.sync.dma_start(rdT[:], rdT_d[:])
            nc.sync.dma_start(rhT[:], rhT_d[:])
            nc.sync.dma_start(rwT[:], rwT_d[:])
            nc.sync.dma_start(bqk[:], bqk_d[:])
            nc.sync.dma_start(bv[:], bv_d[:])

            # ---- V in natural [token, channel] layout, ones column per head ----
            vnat = cpool.tile([128, NKT, HPC * 65], bf16)
            nc.vector.memset(vnat[:], 1.0)
            for kt in range(NKT):
                kp = min(128, N - kt * 128)
                pv = mmp.tile([128, 512], f32, tag="mm")
                for kc in range(NKC):
                    nc.tensor.matmul(
                        pv[0:kp, 0:384],
                        xT[:, kc * N + kt * 128: kc * N + kt * 128 + kp],
                        wqkv[:, kc * 1152 + 768: kc * 1152 + 1152],
                        start=(kc == 0), stop=(kc == NKC - 1),
                    )
                for h6 in range(HPC):
                    nc.vector.tensor_copy(
                        vnat[0:kp, kt, h6 * 65:h6 * 65 + 64],
                        pv[0:kp, h6 * 64:(h6 + 1) * 64],
                    )

            av_all = [
                cpool.tile([128, N], bf16, name=f"av_all{i}", tag=f"av{i}")
                for i in range(3)
            ]

            # ---- head pairs (software-pipelined emission) ----
            def emit_qkv(p):
                augs = []
                for x in range(2):
                    q_t = wpool.tile([KAUG, N], bf16, name=f"qaug{x}", tag=f"qaug{x}")
                    k_t = wpool.tile([128, N], bf16, name=f"kaug{x}", tag=f"kaug{x}")
                    augs.append((q_t, k_t))
                qpair = wpool.tile([128, N], bf16, name="qpair", tag="qpair")
                for qk in range(2):
                    col0 = qk * 384 + p * 128
                    bcol = qk * 3 + p
                    for (f0, fl) in FCH:
                        ps = mmp.tile([128, 512], f32, name="ps", tag="mm")
                        for kc in range(NKC):
                            nc.tensor.matmul(
                                ps[:, 0:fl],
                                wqkv[:, kc * 1152 + col0: kc * 1152 + col0 + 128],
                                xT[:, kc * N + f0: kc * N + f0 + fl],
                                start=(kc == 0), stop=(kc == NKC - 1),
                            )
                        if qk == 0:
                            nc.vector.tensor_scalar_add(
                                qpair[:, f0:f0 + fl], ps[:, 0:fl], bqk[:, bcol:bcol + 1]
                            )
                        else:
                            for x in range(2):
                                nc.vector.tensor_scalar_add(
                                    augs[x][1][0:64, f0:f0 + fl],
                                    ps[x * 64:(x + 1) * 64, 0:fl],
                                    bqk[x * 64:(x + 1) * 64, bcol:bcol + 1],
                                )
                for x in range(2):
                    nc.vector.tensor_copy(
                        augs[x][0][0:64, :], qpair[x * 64:(x + 1) * 64, :]
                    )
                    nc.vector.tensor_copy(augs[x][1][64:KAUG, :], oneh[:])
                return augs, qpair

            def emit_F(p, augs, qpair):
                # F matmuls for BOTH heads at once: block-diagonal stationary
                # puts head A rows at psum 0:nk, head B rows at 32:32+nk.
                qpv = qpair.rearrange("p (d h w) -> p d h w", d=D, h=H, w=W)
                fst = [
                    [wpool.tile([14, N], bf16, name=f"fs{t}{x}", tag=f"fs{t}{x}")
                     for t in range(2)] for x in range(2)
                ]
                views = [
                    [t.rearrange("p (d h w) -> p d h w", d=D, h=H, w=W)
                     for t in row] for row in fst
                ]
                for qd in range(D):
                    pf = mmp.tile([128, 512], f32, name="pf", tag="mm")
                    nc.tensor.matmul(
                        pf[0:40, 0:H * W],
                        rdT[:, qd * 40:(qd + 1) * 40],
                        qpair[:, qd * H * W:(qd + 1) * H * W],
                    )
                    for x in range(2):
                        nc.vector.tensor_copy(
                            augs[x][0][64:72, qd * H * W:(qd + 1) * H * W],
                            pf[x * 32:x * 32 + 8, 0:H * W],
                        )
                for qh in range(H):
                    pf = mmp.tile([128, 512], f32, name="pf", tag="mm")
                    nc.tensor.matmul(
                        pf[0:46, 0:D * W], rhT[:, qh * 46:(qh + 1) * 46], qpv[:, :, qh, :]
                    )
                    for x in range(2):
                        nc.vector.tensor_copy(
                            views[x][0][:, :, qh, :], pf[x * 32:x * 32 + 14, 0:D * W]
                        )
                for qw in range(W):
                    pf = mmp.tile([128, 512], f32, name="pf", tag="mm")
                    nc.tensor.matmul(
                        pf[0:46, 0:D * H], rwT[:, qw * 46:(qw + 1) * 46], qpv[:, :, :, qw]
                    )
                    for x in range(2):
                        nc.vector.tensor_copy(
                            views[x][1][:, :, :, qw], pf[x * 32:x * 32 + 14, 0:D * H]
                        )
                for x in range(2):
                    nc.sync.dma_start(augs[x][0][72:86, :], fst[x][0][:])
                    nc.sync.dma_start(augs[x][0][86:100, :], fst[x][1][:])
                if dbg and p == 0:
                    nc.sync.dma_start(dq_d[:], augs[0][0][0:KAUG, :])
                    nc.sync.dma_start(dk_d[:], augs[0][1][0:KAUG, :])
                    nc.sync.dma_start(dv_d[:], vnat[:].rearrange("p a b -> p (a b)"))

            def emit_att(p, x, augs):
                h6 = 2 * p + x
                q_t, k_t = augs[x]
                pav = avp.tile([65, N], f32, name="pav", tag="av")
                for kt in range(NKT):
                    kp = min(128, N - kt * 128)
                    et = wpool.tile([128, N], bf16, name="et", tag="exp")
                    for (f0, fl) in FCH:
                        ps = mmp.tile([128, 512], f32, name="ps", tag="mm")
                        nc.tensor.matmul(
                            ps[0:kp, 0:fl],
                            k_t[0:KAUG, kt * 128: kt * 128 + kp],
                            q_t[0:KAUG, f0:f0 + fl],
                        )
                        nc.scalar.activation(
                            et[0:kp, f0:f0 + fl], ps[0:kp, 0:fl], AF.Exp
                        )
                    if dbg and h6 == 0 and kt == 0:
                        nc.sync.dma_start(de_d[:], et[:])
                    for (f0, fl) in FCH:
                        nc.tensor.matmul(
                            pav[:, f0:f0 + fl],
                            vnat[0:kp, kt, h6 * 65:(h6 + 1) * 65],
                            et[0:kp, f0:f0 + fl],
                            start=(kt == 0), stop=(kt == NKT - 1),
                        )
                # 1/d = exp(-ln(d)) on the Scalar engine keeps the slow
                # reciprocal off the in-order DVE queue.  For all but the
                # last head, drain PSUM to SBUF first so the next head's
                # AV accumulation can claim the slot immediately.
                last = (p == 2 and x == 1)
                rcp = wpool.tile([33, N], f32, name="rcp", tag="rcp")
                rbc = wpool.tile([64, N], f32, name="rbc", tag="rbc")
                avrows = av_all[p][x * 64:(x + 1) * 64, :]
                if last:
                    nc.scalar.activation(rcp[32:33, :], pav[64:65, :], AF.Ln)
                    nc.scalar.activation(rcp[0:1, :], rcp[32:33, :], AF.Exp, scale=-1.0)
                    nc.gpsimd.partition_broadcast(rbc[0:64, :], rcp[0:1, :])
                    nc.vector.tensor_mul(avrows, pav[0:64, :], rbc[0:64, :])
                else:
                    avst = wpool.tile([65, N], f32, name="avst", tag="avst")
                    nc.vector.tensor_copy(avst[:], pav[:])
                    nc.scalar.activation(rcp[32:33, :], avst[64:65, :], AF.Ln)
                    nc.scalar.activation(rcp[0:1, :], rcp[32:33, :], AF.Exp, scale=-1.0)
                    nc.gpsimd.partition_broadcast(rbc[0:64, :], rcp[0:1, :])
                    nc.vector.tensor_mul(avrows, avst[0:64, :], rbc[0:64, :])
                nc.scalar.activation(
                    avrows, avrows, AF.Identity, bias=bv[:, h6:h6 + 1]
                )
                if dbg and h6 == 0:
                    nc.sync.dma_start(dr_d[:], rbc[:])
                if dbg and p == 0 and x == 1:
                    nc.sync.dma_start(da_d[:], av_all[0][:])

            # pipeline: next pair's qkv between head A and head B, next
            # pair's F after head B — PE always has independent work while
            # DVE/DMA assemble the next pair's tensors.
            cur_augs, cur_qp = emit_qkv(0)
            emit_F(0, cur_augs, cur_qp)
            for p in range(3):
                emit_att(p, 0, cur_augs)
                if p < 2:
                    nxt_augs, nxt_qp = emit_qkv(p + 1)
                emit_att(p, 1, cur_augs)
                if p < 2:
                    emit_F(p + 1, nxt_augs, nxt_qp)
                    cur_augs, cur_qp = nxt_augs, nxt_qp

            # ---- partial projection: outT[768, N] ----
            for mo in range(NKC):
                ot = wpool.tile([128, N], f32, tag="out")
                for (f0, fl) in FCH:
                    ps = mmp.tile([128, 512], f32, tag="mm")
                    for t3 in range(3):
                        nc.tensor.matmul(
                            ps[:, 0:fl],
                            wproj[:, t3 * 768 + mo * 128: t3 * 768 + mo * 128 + 128],
                            av_all[t3][:, f0:f0 + fl],
                            start=(t3 == 0), stop=(t3 == 2),
                        )
                    nc.vector.tensor_copy(ot[:, f0:f0 + fl], ps[:, 0:fl])
                nc.sync.dma_start(out_d[mo], ot[:])

    nc.compile()
    return nc


def _prep_inputs(x, qkv_w, qkv_b, proj_w, proj_b, rel_pos_d, rel_pos_h, rel_pos_w):
    """Host-side shard prep: returns in_maps list for 8 cores."""
    import ml_dtypes
    bf = ml_dtypes.bfloat16
    x = np.ascontiguousarray(x, np.float32)
    qkv_w = np.asarray(qkv_w, np.float32)
    qkv_b = np.asarray(qkv_b, np.float32)
    proj_w = np.asarray(proj_w, np.float32)

    # one-hot k-position rows [36, N]
    j = np.arange(N)
    kd, kh, kw = j // (H * W), (j // W) % H, j % W
    oneh = np.zeros((36, N), np.float32)
    oneh[kd, j] = 1.0
    oneh[8 + kh, j] = 1.0
    oneh[22 + kw, j] = 1.0
    oneh = oneh.astype(bf)

    # rel tables, transposed and un-scaled (q is pre-scaled by SCALE).
    # Block-diagonal over the head pair: head A channels in rows 0:64 feed
    # psum rows 0:n, head B channels in rows 64:128 feed psum rows 32:32+n.
    def rtab(table, n, span):
        t = np.asarray(table, np.float32) / SCALE  # [2n-1, 64]
        qq, kk = np.meshgrid(np.arange(n), np.arange(n), indexing="ij")
        base = t[(qq - kk + n - 1).reshape(-1)].T.reshape(64, n, n)  # [c, q, k]
        out = np.zeros((128, n, span), np.float32)
        out[0:64, :, 0:n] = base.transpose(0, 1, 2)
        out[64:128, :, 32:32 + n] = base
        return np.ascontiguousarray(out.reshape(128, n * span)).astype(bf)

    rdT = rtab(rel_pos_d, D, 40)
    rhT = rtab(rel_pos_h, H, 46)
    rwT = rtab(rel_pos_w, W, 46)

    in_maps = []
    for core in range(8):
        b, g = divmod(core, 2)
        heads = list(range(g * HPC, (g + 1) * HPC))
        # W columns: [q(6x64) | k(6x64) | v(6x64)] for this head group; q scaled
        cols_q = [0 * C + h * HD + c for h in heads for c in range(HD)]
        cols_k = [1 * C + h * HD + c for h in heads for c in range(HD)]
        cols_v = [2 * C + h * HD + c for h in heads for c in range(HD)]
        wq = qkv_w[:, cols_q] * SCALE
        wk = qkv_w[:, cols_k]
        wv = qkv_w[:, cols_v]
        wc = np.concatenate([wq, wk, wv], axis=1)  # [768, 1152]
        wqkv = np.ascontiguousarray(wc.reshape(NKC, 128, 1152)).astype(bf)

        bq = qkv_b[cols_q] * SCALE
        bk = qkv_b[cols_k]
        bvv = qkv_b[cols_v]
        bqk = np.zeros((128, 6), np.float32)
        for p in range(3):
            bqk[:, p] = bq[p * 128:(p + 1) * 128]
            bqk[:, 3 + p] = bk[p * 128:(p + 1) * 128]
        bv_t = np.ascontiguousarray(bvv.reshape(6, 64).T, np.float32)  # [64,6]

        rows = [h * HD + c for h in heads for c in range(HD)]
        wp = np.ascontiguousarray(proj_w[rows].reshape(3, 128, 768)).astype(bf)

        xT = np.ascontiguousarray(
            x[b].reshape(N, C).T.reshape(NKC, 128, N)
        ).astype(bf)
        in_maps.append({
            "xT": xT, "wqkv": wqkv, "wproj": wp, "oneh": oneh,
            "rdT": rdT, "rhT": rhT, "rwT": rwT, "bqk": bqk, "bv": bv_t,
        })
    return in_maps


def _install_ntff_hook_shim():
    """The image's antenv package lacks axon_hooks; recreate it so
    run_bass_kernel_spmd(trace=True) can reach the libaxon NTFF profiler."""
    import types

    if "antenv.axon_hooks" in sys.modules:
        return
    import antenv
    mod = types.ModuleType("antenv.axon_hooks")
    _hook = [None]
    mod.set_axon_ntff_profile_hook = lambda h: _hook.__setitem__(0, h)
    mod.get_axon_ntff_profile_hook = lambda: _hook[0]
    antenv.axon_hooks = mod
    sys.modules["antenv.axon_hooks"] = mod
    try:
        from trn_agent_boot.trn_boot import _ntff_profile_via_ctypes

        mod.set_axon_ntff_profile_hook(
            _ntff_profile_via_ctypes("/opt/axon/libaxon_pjrt.so")
        )
    except Exception as e:  # degrade to no tracing
        print(f"ntff hook shim failed: {e}", file=sys.stderr)


LAST_EXEC_NS = None


def kernel(x, qkv_w, qkv_b, proj_w, proj_b, rel_pos_d, rel_pos_h, rel_pos_w):
    global LAST_EXEC_NS
    if "nc" not in _CACHED:
        _CACHED["nc"] = _build_nc()
    nc = _CACHED["nc"]
    in_maps = _prep_inputs(
        x, qkv_w, qkv_b, proj_w, proj_b, rel_pos_d, rel_pos_h, rel_pos_w
    )
    from concourse.bass_utils import run_bass_kernel_spmd

    trace = bool(int(os.environ.get("KERNEL_TRACE", "0")))
    if trace:
        _install_ntff_hook_shim()
    res = run_bass_kernel_spmd(nc, in_maps, core_ids=list(range(8)), trace=trace)
    LAST_EXEC_NS = res.exec_time_ns
    proj_b = np.asarray(proj_b, np.float32)
    outs = []
    for b in range(B):
        t0 = res.results[2 * b]["out"].reshape(C, N)
        t1 = res.results[2 * b + 1]["out"].reshape(C, N)
        outs.append((t0 + t1).T + proj_b)
    return np.stack(outs).reshape(B, D, H, W, C).astype(np.float32)



# revision 21
# speedup vs baseline: 1.2108x; 1.2108x over previous
"""Distributed Bass kernel: 3D windowed attention with decomposed rel-pos bias.

Sharding: 8 cores = 4 batches x 2 head-groups (6 heads each).
Per-core layout is fully transposed ([channel, token]); the rel-pos bias is
folded into the scores matmul as 36 extra contraction channels (one-hot
k-position rows in the stationary operand, F = q.R tables in the moving
operand).  Softmax runs max-free; the denominator comes free as a ones-row
appended to V in the AV matmul (the ones column and the v bias both ride a
1-row ones-stationary matmul).  All matmuls run in bf16 with fp32 PSUM.

Pipeline: per head the q range [0,1568) is processed in two 784-col halves
so score tiles fit 2 PSUM banks and can double-buffer; emission order per
k-tile is exp(kt), AV(kt), QK(kt+2) which keeps the Scalar engine saturated
with one big exp per tile while the PE runs AV/QK plus interleaved prep
work (next pair's qkv/V/F matmuls) through 2 spare PSUM banks.
"""

import os
import sys
from collections import deque

import numpy as np

sys.path.insert(0, "/opt/trn_rl_repo")

B, D, H, W, C = 4, 8, 14, 14, 768
NH, HD = 12, 64
N = D * H * W  # 1568
HPC = 6  # heads per core
SCALE = HD ** -0.5
NKC = C // 128  # 6 chunks of input channels
NKT = (N + 127) // 128  # 13 k tiles (12x128 + 32)
HALF = 784
CH = [(0, 512), (512, 272)]  # chunks within a 784 half (PSUM bank-sized)
FCH4 = [(0, 512), (512, 512), (1024, 512), (1536, 32)]  # qkv chunks of N
KAUG = 100  # 64 qk channels + 8 + 14 + 14 bias channels

_CACHED = {}


def _build_nc():
    import concourse.bass as bass  # noqa: F401
    import concourse.mybir as mybir
    import concourse.tile as tile
    from concourse import bacc

    f32 = mybir.dt.float32
    bf16 = mybir.dt.bfloat16
    AF = mybir.ActivationFunctionType

    nc = bacc.Bacc(None, target_bir_lowering=False)

    # --- DRAM parameters (per-core shards; host pre-transposes/reorders) ---
    xT_d = nc.declare_dram_parameter("xT", [NKC, 128, N], bf16, isOutput=False)
    wqkv_d = nc.declare_dram_parameter("wqkv", [NKC, 128, 1152], bf16, isOutput=False)
    wproj_d = nc.declare_dram_parameter("wproj", [3, 128, 768], bf16, isOutput=False)
    oneh_d = nc.declare_dram_parameter("oneh", [36, N], bf16, isOutput=False)
    rdT_d = nc.declare_dram_parameter("rdT", [128, D * 40], bf16, isOutput=False)
    rhT_d = nc.declare_dram_parameter("rhT", [128, H * 46], bf16, isOutput=False)
    rwT_d = nc.declare_dram_parameter("rwT", [128, W * 46], bf16, isOutput=False)
    bqk_d = nc.declare_dram_parameter("bqk", [128, 6], f32, isOutput=False)
    bvo_d = nc.declare_dram_parameter("bvo", [1, 3 * 130], bf16, isOutput=False)
    out_d = nc.declare_dram_parameter("out", [NKC, 128, N], bf16, isOutput=True)

    with tile.TileContext(nc) as tc:
        with (
            tc.tile_pool(name="const", bufs=1) as cpool,
            tc.tile_pool(name="work", bufs=2) as wpool,
            tc.tile_pool(name="work1", bufs=1) as w1pool,
            tc.tile_pool(name="scp", bufs=2, space="PSUM") as spool,
            tc.tile_pool(name="pavp", bufs=1, space="PSUM") as pavp,
            tc.tile_pool(name="ppp", bufs=2, space="PSUM") as ppool,
        ):
            # ---- persistent SBUF tensors ----
            xT = cpool.tile([128, NKC * N], bf16)
            wqkv = cpool.tile([128, NKC * 1152], bf16)
            wproj = cpool.tile([128, 3 * 768], bf16)
            oneh = cpool.tile([36, N], bf16)
            rdT = cpool.tile([128, D * 40], bf16)
            rhT = cpool.tile([128, H * 46], bf16)
            rwT = cpool.tile([128, W * 46], bf16)
            bqk = cpool.tile([128, 6], f32)
            bvo = cpool.tile([1, 3 * 130], bf16)
            ones_sb = cpool.tile([1, 128], bf16)
            qaug = [cpool.tile([KAUG, N], bf16, name=f"qaug{h}") for h in range(HPC)]
            kaug = [cpool.tile([KAUG, N], bf16, name=f"kaug{h}") for h in range(HPC)]
            qpair = [cpool.tile([128, N], bf16, name=f"qpair{p}") for p in range(3)]
            vnat = [cpool.tile([128, NKT, 130], bf16, name=f"vnat{p}") for p in range(3)]
            av_all = [cpool.tile([128, N], bf16, name=f"av{p}") for p in range(3)]


            for kc in range(NKC):
                nc.sync.dma_start(xT[:, kc * N:(kc + 1) * N], xT_d[kc])
                nc.sync.dma_start(wqkv[:, kc * 1152:(kc + 1) * 1152], wqkv_d[kc])
            for t3 in range(3):
                nc.sync.dma_start(wproj[:, t3 * 768:(t3 + 1) * 768], wproj_d[t3])
            nc.sync.dma_start(oneh[:], oneh_d[:])
            nc.sync.dma_start(rdT[:], rdT_d[:])
            nc.sync.dma_start(rhT[:], rhT_d[:])
            nc.sync.dma_start(rwT[:], rwT_d[:])
            nc.sync.dma_start(bqk[:], bqk_d[:])
            nc.sync.dma_start(bvo[:], bvo_d[:])
            nc.vector.memset(ones_sb[:], 1.0)
            for h in range(HPC):
                nc.vector.tensor_copy(kaug[h][64:KAUG, :], oneh[:])

            # ---- prep-work generator: qkv + V + F for pair p ----
            def gen_prep(p):
                # qkv q/k projections in 4 chunks of N
                for qk in range(2):
                    for (f0, fl) in FCH4:
                        def qkv_chunk(qk=qk, f0=f0, fl=fl):
                            pt = ppool.tile([128, 512], f32, tag="pp")
                            for kc in range(NKC):
                                nc.tensor.matmul(
                                    pt[:, 0:fl],
                                    wqkv[:, kc * 1152 + p * 384 + qk * 128:
                                         kc * 1152 + p * 384 + qk * 128 + 128],
                                    xT[:, kc * N + f0: kc * N + f0 + fl],
                                    start=(kc == 0), stop=(kc == NKC - 1),
                                )
                            if qk == 0:
                                nc.vector.tensor_scalar_add(
                                    qpair[p][:, f0:f0 + fl], pt[:, 0:fl],
                                    bqk[:, 2 * p:2 * p + 1],
                                )
                                for x in range(2):
                                    nc.vector.tensor_copy(
                                        qaug[2 * p + x][0:64, f0:f0 + fl],
                                        qpair[p][x * 64:(x + 1) * 64, f0:f0 + fl],
                                    )
                            else:
                                for x in range(2):
                                    nc.vector.tensor_scalar_add(
                                        kaug[2 * p + x][0:64, f0:f0 + fl],
                                        pt[x * 64:(x + 1) * 64, 0:fl],
                                        bqk[x * 64:(x + 1) * 64, 2 * p + 1:2 * p + 2],
                                    )
                        yield qkv_chunk
                # V (+ v bias + ones column via 1-row ones stationary)
                for kt in range(NKT):
                    def v_chunk(kt=kt):
                        kp = min(128, N - kt * 128)
                        pt = ppool.tile([128, 512], f32, tag="pp")
                        nc.tensor.matmul(
                            pt[0:kp, 0:130],
                            ones_sb[0:1, 0:kp],
                            bvo[0:1, p * 130:(p + 1) * 130],
                            start=True, stop=False,
                        )
                        for kc in range(NKC):
                            nc.tensor.matmul(
                                pt[0:kp, 0:128],
                                xT[:, kc * N + kt * 128: kc * N + kt * 128 + kp],
                                wqkv[:, kc * 1152 + p * 384 + 256:
                                     kc * 1152 + p * 384 + 384],
                                start=False, stop=(kc == NKC - 1),
                            )
                        vv = vnat[p].rearrange("a t (x c) -> a t x c", x=2)
                        nc.vector.tensor_copy(
                            vv[0:kp, kt, :, 0:64],
                            pt[0:kp, 0:128].rearrange("a (x c) -> a x c", x=2),
                        )
                        nc.vector.tensor_copy(
                            vv[0:kp, kt, :, 64:65],
                            pt[0:kp, 128:130].rearrange("a (x c) -> a x c", x=2),
                        )
                    yield v_chunk
                # F tables. Head A lands at psum rows 0:n, head B at 32:32+n
                # (compute-engine partition windows must start 32-aligned).
                # D's dest rows 64:72 are aligned -> direct copies; H/W dest
                # rows 72:86/86:100 are not -> stage in fst tiles, DMA in.
                qpv = qpair[p].rearrange("a (d h w) -> a d h w", d=D, h=H, w=W)
                fst = [
                    [w1pool.tile([14, N], bf16, name=f"fst{t}{x}", tag=f"fst{t}{x}")
                     for t in range(2)]
                    for x in range(2)
                ]
                fstv = [
                    [t.rearrange("a (d h w) -> a d h w", d=D, h=H, w=W) for t in row]
                    for row in fst
                ]
                # D axis, 2 qd per psum tile (col slots 0 / 256)
                for qd0 in range(0, D, 2):
                    def f_d(qd0=qd0):
                        pt = ppool.tile([128, 512], f32, tag="pp")
                        for j in range(2):
                            nc.tensor.matmul(
                                pt[0:40, j * 256:j * 256 + H * W],
                                rdT[:, (qd0 + j) * 40:(qd0 + j + 1) * 40],
                                qpair[p][:, (qd0 + j) * H * W:(qd0 + j + 1) * H * W],
                            )
                        for x in range(2):
                            dst = qaug[2 * p + x][64:72, qd0 * H * W:(qd0 + 2) * H * W]
                            nc.vector.tensor_copy(
                                dst.rearrange("a (j c) -> a j c", j=2),
                                pt[x * 32:x * 32 + 8, :].rearrange(
                                    "a (j c) -> a j c", j=2)[:, :, 0:H * W],
                            )
                    yield f_d
                # H axis: 4 qh per tile (col slots 0/128/256/384)
                for qh0 in range(0, H, 4):
                    def f_h(qh0=qh0):
                        nq = min(4, H - qh0)
                        pt = ppool.tile([128, 512], f32, tag="pp")
                        for j in range(nq):
                            nc.tensor.matmul(
                                pt[0:46, j * 128:j * 128 + D * W],
                                rhT[:, (qh0 + j) * 46:(qh0 + j + 1) * 46],
                                qpv[:, :, qh0 + j, :],
                            )
                        for x in range(2):
                            nc.vector.tensor_copy(
                                fstv[x][0][:, :, qh0:qh0 + nq, :].rearrange(
                                    "a d h w -> a h d w"),
                                pt[x * 32:x * 32 + 14, 0:nq * 128].rearrange(
                                    "a (j c) -> a j c", j=nq, c=128)[:, :, 0:D * W].rearrange(
                                    "a j (d w) -> a j d w", d=D),
                            )
                    yield f_h
                # W axis
                for qw0 in range(0, W, 4):
                    def f_w(qw0=qw0):
                        nq = min(4, W - qw0)
                        pt = ppool.tile([128, 512], f32, tag="pp")
                        for j in range(nq):
                            nc.tensor.matmul(
                                pt[0:46, j * 128:j * 128 + D * H],
                                rwT[:, (qw0 + j) * 46:(qw0 + j + 1) * 46],
                                qpv[:, :, :, qw0 + j],
                            )
                        for x in range(2):
                            nc.vector.tensor_copy(
                                fstv[x][1][:, :, :, qw0:qw0 + nq].rearrange(
                                    "a d h w -> a w d h"),
                                pt[x * 32:x * 32 + 14, 0:nq * 128].rearrange(
                                    "a (j c) -> a j c", j=nq, c=128)[:, :, 0:D * H].rearrange(
                                    "a j (d h) -> a j d h", d=D),
                            )
                    yield f_w

                def f_dma():
                    for x in range(2):
                        nc.sync.dma_start(qaug[2 * p + x][72:86, :], fst[x][0][:])
                        nc.sync.dma_start(qaug[2 * p + x][86:KAUG, :], fst[x][1][:])
                yield f_dma

            quanta = deque()

            def pull(k=1):
                for _ in range(k):
                    if quanta:
                        quanta.popleft()()

            def drain_quanta():
                while quanta:
                    quanta.popleft()()

            # ---- attention for head h (= 2p + x), one 784-col half ----
            def attn_half(p, x, hf):
                h = 2 * p + x
                q0 = hf * HALF
                pav = pavp.tile([65, HALF], f32, tag="pav")
                scs = {}

                def qk(kt):
                    kp = min(128, N - kt * 128)
                    sc = spool.tile([128, HALF], f32, tag="sc")
                    scs[kt] = sc
                    for (c0, cl) in CH:
                        nc.tensor.matmul(
                            sc[0:kp, c0:c0 + cl],
                            kaug[h][0:KAUG, kt * 128: kt * 128 + kp],
                            qaug[h][0:KAUG, q0 + c0: q0 + c0 + cl],
                        )

                qk(0)
                qk(1)
                for kt in range(NKT):
                    kp = min(128, N - kt * 128)
                    et = wpool.tile([128, HALF], bf16, tag="et")
                    nc.scalar.activation(
                        et[0:kp, 0:HALF], scs[kt][0:kp, 0:HALF], AF.Exp
                    )
                    del scs[kt]
                    for (c0, cl) in CH:
                        nc.tensor.matmul(
                            pav[:, c0:c0 + cl],
                            vnat[p][0:kp, kt, x * 65:(x + 1) * 65],
                            et[0:kp, c0:c0 + cl],
                            start=(kt == 0), stop=(kt == NKT - 1),
                        )
                    if kt + 2 < NKT:
                        qk(kt + 2)
                    pull(1)
                return pav

            stash = {}
            dgat = {}
            for p in range(3):
                if p == 0:
                    for q in gen_prep(0):
                        q()
                    quanta.extend(gen_prep(1))
                else:
                    drain_quanta()  # ensure prep(p) fully emitted pre-attn
                for x in range(2):
                    h = 2 * p + x
                    st = wpool.tile([65, N], f32, tag="stash")
                    stash[h] = st
                    for hf in range(2):
                        pav = attn_half(p, x, hf)
                        nc.vector.tensor_copy(
                            st[:, hf * HALF:(hf + 1) * HALF], pav[:]
                        )
                    dg = wpool.tile([1, N], f32, tag="dgat")
                    dgat[h] = dg
                    nc.sync.dma_start(dg[0:1, :], st[64:65, :])
                # pair boundary: reciprocal of the denominators, then
                # normalize into av_all (runs under next pair's attn)
                for x in range(2):
                    h = 2 * p + x
                    rcp = w1pool.tile([1, N], f32, tag="rcp")
                    rbc = w1pool.tile([64, N], f32, tag="rbc")
                    nc.vector.reciprocal_approx_fast(rcp[0:1, :], dgat[h][0:1, :])
                    nc.gpsimd.partition_broadcast(rbc[0:64, :], rcp[0:1, :])
                    nc.vector.tensor_mul(
                        av_all[p][x * 64:(x + 1) * 64, :], stash[h][0:64, :], rbc[0:64, :]
                    )
                if p == 0:
                    quanta.extend(gen_prep(2))

            # ---- projection: outT[768, N] bf16 partials ----
            for mo in range(NKC):
                ost = wpool.tile([128, N], bf16, tag="ost")
                for (f0, fl) in [(0, HALF), (HALF, HALF)]:
                    pt = spool.tile([128, HALF], f32, tag="sc")
                    for t3 in range(3):
                        for (c0, cl) in CH:
                            nc.tensor.matmul(
                                pt[:, c0:c0 + cl],
                                wproj[:, t3 * 768 + mo * 128: t3 * 768 + mo * 128 + 128],
                                av_all[t3][:, f0 + c0: f0 + c0 + cl],
                                start=(t3 == 0), stop=(t3 == 2),
                            )
                    nc.vector.tensor_copy(ost[:, f0:f0 + HALF], pt[:])
                nc.sync.dma_start(out_d[mo], ost[:])

    nc.compile()
    return nc


def _prep_inputs(x, qkv_w, qkv_b, proj_w, proj_b, rel_pos_d, rel_pos_h, rel_pos_w):
    """Host-side shard prep: returns in_maps list for 8 cores."""
    import ml_dtypes
    bf = ml_dtypes.bfloat16
    x = np.ascontiguousarray(x, np.float32)
    qkv_w = np.asarray(qkv_w, np.float32)
    qkv_b = np.asarray(qkv_b, np.float32)
    proj_w = np.asarray(proj_w, np.float32)

    # one-hot k-position rows [36, N]
    j = np.arange(N)
    kd, kh, kw = j // (H * W), (j // W) % H, j % W
    oneh = np.zeros((36, N), np.float32)
    oneh[kd, j] = 1.0
    oneh[8 + kh, j] = 1.0
    oneh[22 + kw, j] = 1.0
    oneh = oneh.astype(bf)

    # rel tables, transposed and un-scaled (q is pre-scaled by SCALE).
    # Per q-coord block of `span` cols: head A channels (rows 0:64) feed out
    # rows 0:n, head B channels (rows 64:128) feed out rows 32:32+n (psum
    # partition windows must start 32-aligned).
    def rtab(table, n, span):
        t = np.asarray(table, np.float32) / SCALE  # [2n-1, 64]
        out = np.zeros((128, n, span), np.float32)
        for qq in range(n):
            blk = t[qq - np.arange(n) + n - 1].T  # [64 c, n kk]
            out[0:64, qq, 0:n] = blk
            out[64:128, qq, 32:32 + n] = blk
        return np.ascontiguousarray(out.reshape(128, n * span)).astype(bf)

    rdT = rtab(rel_pos_d, D, 40)
    rhT = rtab(rel_pos_h, H, 46)
    rwT = rtab(rel_pos_w, W, 46)

    in_maps = []
    for core in range(8):
        b, g = divmod(core, 2)
        heads = list(range(g * HPC, (g + 1) * HPC))
        # wqkv cols per pair p: [q(128, scaled) | k(128) | v(128)]
        wc = np.zeros((768, 1152), np.float32)
        bqk = np.zeros((128, 6), np.float32)
        bvo = np.zeros((1, 390), np.float32)
        for p in range(3):
            hA, hB = heads[2 * p], heads[2 * p + 1]
            cols2 = [h * HD + c for h in (hA, hB) for c in range(HD)]
            wc[:, p * 384 + 0:p * 384 + 128] = qkv_w[:, [0 * C + c for c in cols2]] * SCALE
            wc[:, p * 384 + 128:p * 384 + 256] = qkv_w[:, [1 * C + c for c in cols2]]
            wc[:, p * 384 + 256:p * 384 + 384] = qkv_w[:, [2 * C + c for c in cols2]]
            bqk[:, 2 * p] = qkv_b[[0 * C + c for c in cols2]] * SCALE
            bqk[:, 2 * p + 1] = qkv_b[[1 * C + c for c in cols2]]
            bvo[0, p * 130:p * 130 + 128] = qkv_b[[2 * C + c for c in cols2]]
            bvo[0, p * 130 + 128:p * 130 + 130] = 1.0
        wqkv = np.ascontiguousarray(wc.reshape(NKC, 128, 1152)).astype(bf)

        rows = [h * HD + c for h in heads for c in range(HD)]
        wp = np.ascontiguousarray(proj_w[rows].reshape(3, 128, 768)).astype(bf)

        xT = np.ascontiguousarray(
            x[b].reshape(N, C).T.reshape(NKC, 128, N)
        ).astype(bf)
        in_maps.append({
            "xT": xT, "wqkv": wqkv, "wproj": wp, "oneh": oneh,
            "rdT": rdT, "rhT": rhT, "rwT": rwT, "bqk": bqk,
            "bvo": bvo.astype(bf),
        })
    return in_maps


def _install_ntff_hook_shim():
    """The image's antenv package lacks axon_hooks; recreate it so
    run_bass_kernel_spmd(trace=True) can reach the libaxon NTFF profiler."""
    import types

    if "antenv.axon_hooks" in sys.modules:
        return
    import antenv
    mod = types.ModuleType("antenv.axon_hooks")
    _hook = [None]
    mod.set_axon_ntff_profile_hook = lambda h: _hook.__setitem__(0, h)
    mod.get_axon_ntff_profile_hook = lambda: _hook[0]
    antenv.axon_hooks = mod
    sys.modules["antenv.axon_hooks"] = mod
    try:
        from trn_agent_boot.trn_boot import _ntff_profile_via_ctypes

        mod.set_axon_ntff_profile_hook(
            _ntff_profile_via_ctypes("/opt/axon/libaxon_pjrt.so")
        )
    except Exception as e:  # degrade to no tracing
        print(f"ntff hook shim failed: {e}", file=sys.stderr)


LAST_EXEC_NS = None


def kernel(x, qkv_w, qkv_b, proj_w, proj_b, rel_pos_d, rel_pos_h, rel_pos_w):
    global LAST_EXEC_NS
    if "nc" not in _CACHED:
        _CACHED["nc"] = _build_nc()
    nc = _CACHED["nc"]
    in_maps = _prep_inputs(
        x, qkv_w, qkv_b, proj_w, proj_b, rel_pos_d, rel_pos_h, rel_pos_w
    )
    from concourse.bass_utils import run_bass_kernel_spmd

    trace = bool(int(os.environ.get("KERNEL_TRACE", "0")))
    if trace:
        _install_ntff_hook_shim()
    res = run_bass_kernel_spmd(nc, in_maps, core_ids=list(range(8)), trace=trace)
    LAST_EXEC_NS = res.exec_time_ns
    proj_b = np.asarray(proj_b, np.float32)
    outs = []
    for b in range(B):
        t0 = res.results[2 * b]["out"].astype(np.float32).reshape(C, N)
        t1 = res.results[2 * b + 1]["out"].astype(np.float32).reshape(C, N)
        outs.append((t0 + t1).T + proj_b)
    return np.stack(outs).reshape(B, D, H, W, C).astype(np.float32)
